# revision 20
# baseline (speedup 1.0000x reference)
"""Causal self-attention (B=4, T=2048, C=768, H=12) on 8 TRN2 NeuronCores.

Sharding: (batch x head-half). Core c handles batch b = c//2 and heads
hg*6..hg*6+5 where hg = c%2. Each core computes the qkv projection for its
1152 W_attn columns, causal attention for its 6 heads, and a partial
c_proj using its 384 rows of W_proj. Host sums the pair partials + b_eff.

v3 structure (software-pipelined single stream):
- Query chunks processed in order [0, 3, 2, 1] so the exp-heavy chunks are
  not last (ScalarE exp is the secondary bottleneck; the last chunk's exp
  tail would leave the PE idle).
- Q-proj / K-proj / V-proj / c_proj are decoupled into small work items
  that are emitted just-in-time before the attention pti that needs them,
  or earlier as "fillers" paced into the ACT-bound attention inner loop so
  the PE never stalls while ScalarE streams exp.
- Score matmuls of a head pair are emitted back-to-back (h2=0 on PE rows
  0-63, h2=1 on rows 64-127) so they run concurrently in the PE array.
- Bias algebra: K-projection bias dropped (softmax-invariant), V bias
  folded host-side into b_eff = b_proj + b_v @ W_proj, Q bias enters as
  exp(bqK/8) folded into the PV stationary operand.
- PV stationary carries the scaled-ones column (row 64 of the PV
  accumulator = softmax denominator).
- c_proj is split into two 384-wide halves (1 PSUM bank each) and deferred
  one chunk, emitted as filler work.
"""

import sys

import numpy as np

try:
    import concourse  # noqa: F401
except ImportError:
    sys.path.insert(0, "/opt/trn_rl_repo")

B, T, C, H, D = 4, 2048, 768, 12, 64
NH = H // 2          # 6 heads per core
CH = NH * D          # 384 channels per core
NCB = C // 128       # 6 contraction blocks
NTB = T // 128       # 16 t-blocks
NQC = T // 512       # 4 query chunks
NPAIR = NH // 2      # 3 head pairs
VW2 = D + 2          # 66: [V(64), eb, pad] per head (col 64 = eb = exp(bqK/8))
VROW = NH * VW2      # 396
CORDER = [0, 3, 2, 1]

_CACHE = {}


def _build_nc():
    from concourse import bacc, mybir, tile

    f32 = mybir.dt.float32
    bf16 = mybir.dt.bfloat16
    AF = mybir.ActivationFunctionType
    ALU = mybir.AluOpType

    nc = bacc.Bacc("TRN2", target_bir_lowering=False, debug=False, num_devices=8)

    xt_d = nc.dram_tensor("xt", [C, T], bf16, kind="ExternalInput")
    wqk_d = nc.dram_tensor("wqk", [C, 2 * CH], bf16, kind="ExternalInput")
    wv_d = nc.dram_tensor("wv", [C, CH + NH], bf16, kind="ExternalInput")
    wp_d = nc.dram_tensor("wp", [128, NPAIR * C], bf16, kind="ExternalInput")
    out_d = nc.dram_tensor("out", [T, C], f32, kind="ExternalOutput")

    with tile.TileContext(nc) as tc:
        with (
            tc.tile_pool(name="const", bufs=1) as cp,
            tc.tile_pool(name="wk", bufs=3) as wk,
            tc.tile_pool(name="pt", bufs=3) as ptp,
            tc.tile_pool(name="ot", bufs=2) as otp,
            tc.tile_pool(name="outs", bufs=2) as osp,
            tc.tile_pool(name="ps", bufs=2, space="PSUM") as psS,
            tc.tile_pool(name="pj", bufs=2, space="PSUM") as psP,
            tc.tile_pool(name="pv", bufs=2, space="PSUM") as psV,
        ):
            # ---- resident inputs (full-width rows: max DMA run length) ----
            xt_r = xt_d.rearrange("(n p) m -> n p m", p=128)
            wqk_r = wqk_d.rearrange("(n p) m -> n p m", p=128)
            wv_r = wv_d.rearrange("(n p) m -> n p m", p=128)
            xt_t, wqk_t, wv_t = [], [], []
            for ci in range(NCB):
                t_ = cp.tile([128, T], bf16, tag=f"xt{ci}", name=f"xt{ci}")
                nc.sync.dma_start(out=t_, in_=xt_r[ci])
                xt_t.append(t_)
                t_ = cp.tile([128, 2 * CH], bf16, tag=f"wqk{ci}", name=f"wqk{ci}")
                nc.sync.dma_start(out=t_, in_=wqk_r[ci])
                wqk_t.append(t_)
            for ci in range(NCB):
                t_ = cp.tile([128, CH + NH], bf16, tag=f"wv{ci}", name=f"wv{ci}")
                nc.sync.dma_start(out=t_, in_=wv_r[ci])
                wv_t.append(t_)
            wp_sb = cp.tile([128, NPAIR, C], bf16, tag="wp", name="wp")
            nc.sync.dma_start(out=wp_sb, in_=wp_d.rearrange("p (n m) -> p n m", n=NPAIR))

            qkT = cp.tile([128, 6, T], bf16, tag="qkT", name="qkT")  # 0-2: Q, 3-5: K
            v1 = cp.tile([128, NTB, VROW], bf16, tag="v1", name="v1")
            v1_4d = v1.rearrange("p n (h e) -> p n h e", e=VW2)

            # ---------- work items ----------
            done_qk = {}       # (co, tc) -> True   co 0-2 Q-pair, 3-5 K-pair
            done_v = {}        # tb -> True

            def emit_proj(co, tcn):
                """Q or K projection for pair-column co, token chunk tcn."""
                if done_qk.get((co, tcn)):
                    return 0
                done_qk[(co, tcn)] = True
                ps = psP.tile([128, 512], f32, tag="pj", name="pspj")
                for ci in range(NCB):
                    nc.tensor.matmul(
                        ps,
                        lhsT=wqk_t[ci][:, co * 128:(co + 1) * 128],
                        rhs=xt_t[ci][:, tcn * 512:(tcn + 1) * 512],
                        start=(ci == 0),
                        stop=(ci == NCB - 1),
                    )
                nc.vector.tensor_copy(qkT[:, co, tcn * 512:(tcn + 1) * 512], ps)
                return 1450

            def emit_v(tb):
                """V (+ bqK) projection for key t-block tb."""
                if done_v.get(tb):
                    return 0
                done_v[tb] = True
                psv = psP.tile([128, 512], f32, tag="pj", name="pspj")
                for ci in range(NCB):
                    nc.tensor.matmul(
                        psv[:, 0:CH + NH],
                        lhsT=xt_t[ci][:, tb * 128:(tb + 1) * 128],
                        rhs=wv_t[ci],
                        start=(ci == 0),
                        stop=(ci == NCB - 1),
                    )
                eb = wk.tile([128, NH], f32, tag="eb", name="eb")
                nc.scalar.activation(eb, psv[:, CH:CH + NH], AF.Exp, scale=0.125)
                eb3 = eb.rearrange("p (h o) -> p h o", o=1)
                nc.vector.tensor_mul(
                    v1_4d[:, tb, :, 0:D],
                    psv[:, 0:CH].rearrange("p (h e) -> p h e", e=D),
                    eb3.to_broadcast([128, NH, D]),
                )
                nc.vector.tensor_copy(v1_4d[:, tb, :, D:D + 1], eb3)
                return 1150

            def emit_cproj_half(c, tb4, half, ot_tiles):
                """c_proj for t-block c*4+tb4, output columns half*384:+384."""
                tb = c * 4 + tb4
                pp = psP.tile([128, 512], f32, tag="pj", name="pspj")
                for p in range(NPAIR):
                    nc.tensor.matmul(
                        pp[:, 0:CH],
                        lhsT=ot_tiles[p][:, tb4 * 128:(tb4 + 1) * 128],
                        rhs=wp_sb[:, p, half * CH:(half + 1) * CH],
                        start=(p == 0),
                        stop=(p == NPAIR - 1),
                    )
                ost = osp.tile([128, CH], f32, tag=f"ost{half}", name="ost")
                nc.vector.tensor_copy(ost, pp[:, 0:CH])
                nc.sync.dma_start(
                    out=out_d[tb * 128:(tb + 1) * 128, half * CH:(half + 1) * CH],
                    in_=ost,
                )
                return 680

            cfill = []         # c_proj closures (must drain one chunk ahead)
            fillers = []       # proj/V closures returning pe-ns
            debt = [0.0]
            ot_map = {}        # chunk -> [otpair per pair]
            norm_pending = [None]

            def flush_norm():
                if norm_pending[0] is None:
                    return
                ps_pv, ots, p = norm_pending[0]
                norm_pending[0] = None
                otpair = otp.tile([128, 512], bf16, tag=f"ot{p}",
                                  name=f"ot{p}")
                for h2 in range(2):
                    # sums live at partition 64; custom-DVE ops misread
                    # base-64 APs, so standard-copy to partition 0 first
                    sums_sb = wk.tile([1, 512], f32, tag="sums", name="sums")
                    nc.vector.tensor_copy(sums_sb, ps_pv[h2][D:D + 1, :])
                    rb1 = wk.tile([1, 512], f32, tag="rb1", name="rb1")
                    nc.vector.reciprocal_approx_fast(rb1, sums_sb)
                    rbb = wk.tile([64, 512], f32, tag="rbb", name="rbb")
                    nc.gpsimd.partition_broadcast(rbb, rb1)
                    nc.vector.tensor_mul(
                        otpair[h2 * 64:(h2 + 1) * 64, :],
                        ps_pv[h2][0:D, :],
                        rbb,
                    )
                ots[p] = otpair

            def pump(ns):
                debt[0] += ns
                while debt[0] > 0 and (cfill or fillers):
                    q = cfill if cfill else fillers
                    debt[0] -= q.pop(0)()

            def emit_pv(nc_, pts, ps_pv, p, pti, j, nkb):
                for half, kb in ((0, 2 * pti), (1, 2 * pti + 1)):
                    pt, wd = pts[half]
                    qlo = 512 - wd
                    for h2 in range(2):
                        nc_.tensor.matmul(
                            ps_pv[h2][0:1 + D, qlo:512],
                            lhsT=v1_4d[:, kb, 2 * p + h2, 0:1 + D],
                            rhs=pt[:, h2 * 512:h2 * 512 + wd],
                            start=(kb == 0),
                            stop=(kb == nkb - 1),
                        )

            # ---------- main stream ----------
            pending = None     # (chunk, ot_tiles) whose c_proj is deferred
            for ic, j in enumerate(CORDER):
                nkb = 4 * (j + 1)
                # c_proj fillers of the chunk-before-last MUST be fully
                # emitted before this chunk's normalization recycles the ot
                # buffers (otp bufs=2), else PE-queue/WAR cycle -> deadlock
                for f in cfill:
                    f()
                cfill.clear()
                # queue fillers: deferred c_proj, then next chunk's prereqs
                if pending is not None:
                    pc, pots = pending
                    for tb4 in range(4):
                        for half in range(2):
                            cfill.append(
                                lambda c=pc, t=tb4, hf=half, o=pots:
                                emit_cproj_half(c, t, hf, o)
                            )
                    pending = None
                if ic + 1 < len(CORDER):
                    cn = CORDER[ic + 1]
                    for p in range(NPAIR):
                        for tcn in range(cn + 1):
                            if not done_qk.get((3 + p, tcn)):
                                fillers.append(
                                    lambda co=3 + p, t=tcn: emit_proj(co, t))
                        if not done_qk.get((p, cn)):
                            fillers.append(lambda co=p, t=cn: emit_proj(co, t))
                    for tb in range(4 * (cn + 1)):
                        if not done_v.get(tb):
                            fillers.append(lambda t=tb: emit_v(t))

                ot_map[j] = [None] * NPAIR
                for p in range(NPAIR):
                    # jit prereqs for this pair (K-proj for later key chunks
                    # is deferred into the pti loop to fill the ACT deficit)
                    debt[0] -= emit_proj(p, j)
                    debt[0] -= emit_proj(3 + p, 0)
                    flush_norm()
                    ps_pv = [
                        psV.tile([128, 512], f32, tag="pv", name=f"pspv{h2}")
                        for h2 in range(2)
                    ]
                    prev = None
                    pw = 1024.0
                    for pti in range(nkb // 2):
                        kb0, kb1 = 2 * pti, 2 * pti + 1
                        debt[0] -= emit_proj(3 + p, kb1 // 4)
                        if kb1 // 4 + 1 <= j and pti % 2 == 1:
                            # prefetch next key chunk's K-proj one pti early
                            debt[0] -= emit_proj(3 + p, kb1 // 4 + 1)
                        d0 = kb0 * 128 - j * 512
                        d1 = d0 + 128
                        qlo0, qlo1 = max(d0, 0), max(d1, 0)
                        w0, w1 = 512 - qlo0, 512 - qlo1
                        # one psum tile per key block holding BOTH h2 halves
                        # (h2=1 at column 512) so the paired score matmuls
                        # release together and issue back-to-back
                        pss = [
                            psS.tile([128, 1024], f32, tag="s", name=f"pss{kk}")
                            for kk in range(2)
                        ]
                        for kk, (kb, wd, ql) in enumerate(
                                ((kb0, w0, qlo0), (kb1, w1, qlo1))):
                            for h2 in range(2):
                                hp = h2 * 64
                                nc.tensor.matmul(
                                    pss[kk][:, h2 * 512:h2 * 512 + wd],
                                    lhsT=qkT[hp:hp + 64, 3 + p,
                                             kb * 128:(kb + 1) * 128],
                                    rhs=qkT[hp:hp + 64, p,
                                            j * 512 + ql:(j + 1) * 512],
                                    start=True,
                                    stop=True,
                                )
                        cur = []
                        for kk, (kb, wd, dd) in enumerate(
                                ((kb0, w0, d0), (kb1, w1, d1))):
                            pt = ptp.tile([128, 1024], bf16, tag=f"pt{kk}",
                                          name=f"pt{kk}")
                            if wd == 512:
                                nc.scalar.activation(
                                    pt, pss[kk], AF.Exp, scale=0.125)
                            else:
                                for h2 in range(2):
                                    nc.scalar.activation(
                                        pt[:, h2 * 512:h2 * 512 + wd],
                                        pss[kk][:, h2 * 512:h2 * 512 + wd],
                                        AF.Exp, scale=0.125,
                                    )
                            if dd >= 0:
                                for h2 in range(2):
                                    nc.gpsimd.affine_select(
                                        out=pt[:, h2 * 512:h2 * 512 + 128],
                                        in_=pt[:, h2 * 512:h2 * 512 + 128],
                                        compare_op=ALU.is_ge, fill=0.0, base=0,
                                        pattern=[[1, 128]],
                                        channel_multiplier=-1,
                                    )
                            cur.append((pt, wd))
                        # V for these key blocks (consumed by NEXT pti's PV):
                        # emitted after the scores so the exp stream is never
                        # blocked behind V-proj (which gates on the wv DMA)
                        debt[0] -= emit_v(kb0)
                        debt[0] -= emit_v(kb1)
                        # deficit: ACT exp time minus attention PE time this pti
                        sw = w0 + w1
                        if prev is not None:
                            emit_pv(nc, prev, ps_pv, p, pti - 1, j, nkb)
                            pump(1.25 * sw + 358 - 0.833 * pw)
                        else:
                            pump(1.25 * sw + 358)
                        pw = sw
                        prev = cur
                    emit_pv(nc, prev, ps_pv, p, nkb // 2 - 1, j, nkb)
                    # normalization is deferred one pair (flushed after the
                    # NEXT pair's jit projections) so its vector-queue ops
                    # never block the qkT casts the next scores need
                    norm_pending[0] = (ps_pv, ot_map[j], p)
                pending = (j, ot_map[j])

            # drain leftovers, then the last chunk's c_proj
            flush_norm()
            for f in cfill:
                f()
            for f in fillers:
                f()
            pc, pots = pending
            for tb4 in range(4):
                for half in range(2):
                    emit_cproj_half(pc, tb4, half, pots)

    nc.compile()
    return nc


def _bf16(a):
    import ml_dtypes
    return np.ascontiguousarray(a).astype(ml_dtypes.bfloat16)


def _shard_inputs(x, W_attn, b_attn, W_proj):
    in_maps = []
    for c in range(8):
        b, hg = c // 2, c % 2
        q0, k0, v0 = hg * CH, C + hg * CH, 2 * C + hg * CH
        # per-head bqK column: (Wk_h @ bq_h) -> scores bias via exp-fold
        bcols = np.stack(
            [
                W_attn[:, k0 + h * D:k0 + (h + 1) * D]
                @ b_attn[q0 + h * D:q0 + (h + 1) * D]
                for h in range(NH)
            ],
            axis=1,
        )  # [C, 6]
        in_maps.append({
            "xt": _bf16(x[b].T),
            "wqk": _bf16(np.concatenate(
                [W_attn[:, q0:q0 + CH], W_attn[:, k0:k0 + CH]], axis=1)),
            "wv": _bf16(np.concatenate(
                [W_attn[:, v0:v0 + CH], bcols], axis=1)),
            "wp": _bf16(
                W_proj[hg * CH:(hg + 1) * CH, :]
                .reshape(NPAIR, 128, C)
                .transpose(1, 0, 2)
                .reshape(128, NPAIR * C)
            ),
        })
    return in_maps


def kernel(x, W_attn, b_attn, W_proj, b_proj, _trace=False):
    from concourse.bass_utils import run_bass_kernel_spmd

    x = np.asarray(x, dtype=np.float32)
    W_attn = np.asarray(W_attn, dtype=np.float32)
    b_attn = np.asarray(b_attn, dtype=np.float32)
    W_proj = np.asarray(W_proj, dtype=np.float32)
    b_proj = np.asarray(b_proj, dtype=np.float32)

    if "nc" not in _CACHE:
        _CACHE["nc"] = _build_nc()
    nc = _CACHE["nc"]

    in_maps = _shard_inputs(x, W_attn, b_attn, W_proj)
    res = run_bass_kernel_spmd(nc, in_maps, list(range(8)), trace=_trace)
    _CACHE["last_result"] = res

    # V-bias contribution is a constant row: b_eff = b_proj + b_v @ W_proj
    b_eff = b_proj + b_attn[2 * C:] @ W_proj
    out = np.empty((B, T, C), dtype=np.float32)
    for b in range(B):
        out[b] = res.results[2 * b]["out"] + res.results[2 * b + 1]["out"] + b_eff
    return out


# revision 24
# speedup vs baseline: 1.1527x; 1.1527x over previous
"""Causal self-attention (B=4, T=2048, C=768, H=12) on 8 TRN2 NeuronCores.

Sharding: (batch x head-half). Core c handles batch b = c//2 and heads
hg*6..hg*6+5 where hg = c%2. Each core computes the qkv projection for its
1152 W_attn columns, causal attention for its 6 heads, and a partial
c_proj using its 384 rows of W_proj. Host sums the pair partials + b_eff.

v3 structure (software-pipelined single stream):
- Query chunks processed in order [0, 3, 2, 1] so the exp-heavy chunks are
  not last (ScalarE exp is the secondary bottleneck; the last chunk's exp
  tail would leave the PE idle).
- Q-proj / K-proj / V-proj / c_proj are decoupled into small work items
  that are emitted just-in-time before the attention pti that needs them,
  or earlier as "fillers" paced into the ACT-bound attention inner loop so
  the PE never stalls while ScalarE streams exp.
- Score matmuls of a head pair are emitted back-to-back (h2=0 on PE rows
  0-63, h2=1 on rows 64-127) so they run concurrently in the PE array.
- Bias algebra: K-projection bias dropped (softmax-invariant), V bias
  folded host-side into b_eff = b_proj + b_v @ W_proj, Q bias enters as
  exp(bqK/8) folded into the PV stationary operand.
- PV stationary carries the scaled-ones column (row 64 of the PV
  accumulator = softmax denominator).
- c_proj is split into two 384-wide halves (1 PSUM bank each) and deferred
  one chunk, emitted as filler work.
"""

import sys

import numpy as np

try:
    import concourse  # noqa: F401
except ImportError:
    sys.path.insert(0, "/opt/trn_rl_repo")

B, T, C, H, D = 4, 2048, 768, 12, 64
NH = H // 2          # 6 heads per core
CH = NH * D          # 384 channels per core
NCB = C // 128       # 6 contraction blocks
NTB = T // 128       # 16 t-blocks
NQC = T // 512       # 4 query chunks
NPAIR = NH // 2      # 3 head pairs
VW2 = D + 2          # 66: [V(64), eb, pad] per head (col 64 = eb = exp(bqK/8))
VROW = NH * VW2      # 396
CORDER = [0, 3, 2, 1]

_CACHE = {}


def _build_nc():
    from concourse import bacc, mybir, tile

    f32 = mybir.dt.float32
    bf16 = mybir.dt.bfloat16
    AF = mybir.ActivationFunctionType
    ALU = mybir.AluOpType

    nc = bacc.Bacc("TRN2", target_bir_lowering=False, debug=False, num_devices=8)

    xt_d = nc.dram_tensor("xt", [C, T], bf16, kind="ExternalInput")
    wqk_d = nc.dram_tensor("wqk", [C, 2 * CH], bf16, kind="ExternalInput")
    wv_d = nc.dram_tensor("wv", [C, CH + NH], bf16, kind="ExternalInput")
    wp_d = nc.dram_tensor("wp", [128, NPAIR * C], bf16, kind="ExternalInput")
    out_d = nc.dram_tensor("out", [T, C], f32, kind="ExternalOutput")

    with tile.TileContext(nc) as tc:
        with (
            tc.tile_pool(name="const", bufs=1) as cp,
            tc.tile_pool(name="wk", bufs=3) as wk,
            tc.tile_pool(name="pt", bufs=3) as ptp,
            tc.tile_pool(name="ot", bufs=2) as otp,
            tc.tile_pool(name="outs", bufs=2) as osp,
            tc.tile_pool(name="ps", bufs=2, space="PSUM") as psS,
            tc.tile_pool(name="pj", bufs=2, space="PSUM") as psP,
            tc.tile_pool(name="pv", bufs=2, space="PSUM") as psV,
        ):
            # ---- resident inputs (full-width rows: max DMA run length) ----
            xt_r = xt_d.rearrange("(n p) m -> n p m", p=128)
            wqk_r = wqk_d.rearrange("(n p) m -> n p m", p=128)
            wv_r = wv_d.rearrange("(n p) m -> n p m", p=128)
            xt_t, wqk_t, wv_t = [], [], []
            for ci in range(NCB):
                t_ = cp.tile([128, T], bf16, tag=f"xt{ci}", name=f"xt{ci}")
                nc.sync.dma_start(out=t_, in_=xt_r[ci])
                xt_t.append(t_)
                t_ = cp.tile([128, 2 * CH], bf16, tag=f"wqk{ci}", name=f"wqk{ci}")
                nc.sync.dma_start(out=t_, in_=wqk_r[ci])
                wqk_t.append(t_)
            for ci in range(NCB):
                t_ = cp.tile([128, CH + NH], bf16, tag=f"wv{ci}", name=f"wv{ci}")
                nc.sync.dma_start(out=t_, in_=wv_r[ci])
                wv_t.append(t_)
            wp_sb = cp.tile([128, NPAIR, C], bf16, tag="wp", name="wp")
            nc.sync.dma_start(out=wp_sb, in_=wp_d.rearrange("p (n m) -> p n m", n=NPAIR))

            qkT = cp.tile([128, 6, T], bf16, tag="qkT", name="qkT")  # 0-2: Q, 3-5: K
            v1 = cp.tile([128, NTB, VROW], bf16, tag="v1", name="v1")
            v1_4d = v1.rearrange("p n (h e) -> p n h e", e=VW2)

            # ---------- work items ----------
            done_qk = {}       # (co, tc) -> True   co 0-2 Q-pair, 3-5 K-pair
            done_v = {}        # tb -> True

            def emit_proj(co, tcn):
                """Q or K projection for pair-column co, token chunk tcn."""
                if done_qk.get((co, tcn)):
                    return 0
                done_qk[(co, tcn)] = True
                ps = psP.tile([128, 512], f32, tag="pj", name="pspj")
                for ci in range(NCB):
                    nc.tensor.matmul(
                        ps,
                        lhsT=wqk_t[ci][:, co * 128:(co + 1) * 128],
                        rhs=xt_t[ci][:, tcn * 512:(tcn + 1) * 512],
                        start=(ci == 0),
                        stop=(ci == NCB - 1),
                    )
                nc.vector.tensor_copy(qkT[:, co, tcn * 512:(tcn + 1) * 512], ps)
                return 1450

            def emit_v(tb):
                """V (+ bqK) projection for key t-block tb."""
                if done_v.get(tb):
                    return 0
                done_v[tb] = True
                psv = psP.tile([128, 512], f32, tag="pj", name="pspj")
                for ci in range(NCB):
                    nc.tensor.matmul(
                        psv[:, 0:CH + NH],
                        lhsT=xt_t[ci][:, tb * 128:(tb + 1) * 128],
                        rhs=wv_t[ci],
                        start=(ci == 0),
                        stop=(ci == NCB - 1),
                    )
                eb = wk.tile([128, NH], f32, tag="eb", name="eb")
                nc.scalar.activation(eb, psv[:, CH:CH + NH], AF.Exp, scale=0.125)
                eb3 = eb.rearrange("p (h o) -> p h o", o=1)
                nc.vector.tensor_mul(
                    v1_4d[:, tb, :, 0:D],
                    psv[:, 0:CH].rearrange("p (h e) -> p h e", e=D),
                    eb3.to_broadcast([128, NH, D]),
                )
                nc.vector.tensor_copy(v1_4d[:, tb, :, D:D + 1], eb3)
                return 1150

            def emit_cproj_half(c, tb4, half, ot_tiles):
                """c_proj for t-block c*4+tb4, output columns half*384:+384."""
                tb = c * 4 + tb4
                pp = psP.tile([128, 512], f32, tag="pj", name="pspj")
                for p in range(NPAIR):
                    nc.tensor.matmul(
                        pp[:, 0:CH],
                        lhsT=ot_tiles[p][:, tb4 * 128:(tb4 + 1) * 128],
                        rhs=wp_sb[:, p, half * CH:(half + 1) * CH],
                        start=(p == 0),
                        stop=(p == NPAIR - 1),
                    )
                ost = osp.tile([128, CH], f32, tag=f"ost{half}", name="ost")
                nc.vector.tensor_copy(ost, pp[:, 0:CH])
                nc.sync.dma_start(
                    out=out_d[tb * 128:(tb + 1) * 128, half * CH:(half + 1) * CH],
                    in_=ost,
                )
                return 680

            cfill = []         # c_proj closures (must drain one chunk ahead)
            fillers = []       # proj/V closures returning pe-ns
            debt = [0.0]
            ot_map = {}        # chunk -> [otpair per pair]
            norm_pending = [None]

            def flush_norm():
                if norm_pending[0] is None:
                    return
                ps_pv, ots, p = norm_pending[0]
                norm_pending[0] = None
                otpair = otp.tile([128, 512], bf16, tag=f"ot{p}",
                                  name=f"ot{p}")
                for h2 in range(2):
                    # sums live at partition 64; custom-DVE ops misread
                    # base-64 APs, so standard-copy to partition 0 first
                    sums_sb = wk.tile([1, 512], f32, tag="sums", name="sums")
                    nc.vector.tensor_copy(sums_sb, ps_pv[h2][D:D + 1, :])
                    rb1 = wk.tile([1, 512], f32, tag="rb1", name="rb1")
                    nc.vector.reciprocal_approx_fast(rb1, sums_sb)
                    rbb = wk.tile([64, 512], f32, tag="rbb", name="rbb")
                    nc.gpsimd.partition_broadcast(rbb, rb1)
                    nc.vector.tensor_mul(
                        otpair[h2 * 64:(h2 + 1) * 64, :],
                        ps_pv[h2][0:D, :],
                        rbb,
                    )
                ots[p] = otpair

            def pump(ns):
                debt[0] += ns
                while debt[0] > 0 and (cfill or fillers):
                    q = cfill if cfill else fillers
                    debt[0] -= q.pop(0)()

            def emit_pv(nc_, pts, ps_pv, p, pti, j, nkb):
                for half, kb in ((0, 2 * pti), (1, 2 * pti + 1)):
                    pt, wd = pts[half]
                    qlo = 512 - wd
                    for h2 in range(2):
                        nc_.tensor.matmul(
                            ps_pv[h2][0:1 + D, qlo:512],
                            lhsT=v1_4d[:, kb, 2 * p + h2, 0:1 + D],
                            rhs=pt[:, h2 * 512:h2 * 512 + wd],
                            start=(kb == 0),
                            stop=(kb == nkb - 1),
                        )

            # last chunk's c_proj is split per pair: partials accumulate in
            # SBUF so only the p=2 matmuls trail the final normalization
            lastc = CORDER[-1]
            accs = {}
            for tb4 in range(4):
                for half in range(2):
                    accs[(tb4, half)] = osp.tile(
                        [128, CH], f32, tag=f"acc{tb4}_{half}", bufs=1,
                        name="acc")

            def cpart(tb4, half, p):
                pp = psP.tile([128, 512], f32, tag="pj", name="pspj")
                nc.tensor.matmul(
                    pp[:, 0:CH],
                    lhsT=ot_map[lastc][p][:, tb4 * 128:(tb4 + 1) * 128],
                    rhs=wp_sb[:, p, half * CH:(half + 1) * CH],
                    start=True,
                    stop=True,
                )
                acc = accs[(tb4, half)]
                if p == 0:
                    nc.vector.tensor_copy(acc, pp[:, 0:CH])
                else:
                    nc.vector.tensor_add(acc, acc, pp[:, 0:CH])
                if p == NPAIR - 1:
                    tb = lastc * 4 + tb4
                    nc.sync.dma_start(
                        out=out_d[tb * 128:(tb + 1) * 128,
                                  half * CH:(half + 1) * CH],
                        in_=acc,
                    )
                return 400

            # ---------- main stream ----------
            pending = None     # (chunk, ot_tiles) whose c_proj is deferred
            for ic, j in enumerate(CORDER):
                nkb = 4 * (j + 1)
                # c_proj fillers of the chunk-before-last MUST be fully
                # emitted before this chunk's normalization recycles the ot
                # buffers (otp bufs=2), else PE-queue/WAR cycle -> deadlock
                for f in cfill:
                    f()
                cfill.clear()
                # queue fillers: deferred c_proj, then next chunk's prereqs
                if pending is not None:
                    pc, pots = pending
                    for tb4 in range(4):
                        for half in range(2):
                            cfill.append(
                                lambda c=pc, t=tb4, hf=half, o=pots:
                                emit_cproj_half(c, t, hf, o)
                            )
                    pending = None
                if ic + 1 < len(CORDER):
                    cn = CORDER[ic + 1]
                    for p in range(NPAIR):
                        for tcn in range(cn + 1):
                            if not done_qk.get((3 + p, tcn)):
                                fillers.append(
                                    lambda co=3 + p, t=tcn: emit_proj(co, t))
                        if not done_qk.get((p, cn)):
                            fillers.append(lambda co=p, t=cn: emit_proj(co, t))
                    for tb in range(4 * (cn + 1)):
                        if not done_v.get(tb):
                            fillers.append(lambda t=tb: emit_v(t))

                ot_map[j] = [None] * NPAIR
                for p in range(NPAIR):
                    # jit prereqs for this pair
                    debt[0] -= emit_proj(p, j)
                    for tcn in range(j + 1):
                        debt[0] -= emit_proj(3 + p, tcn)
                    flush_norm()
                    if j == lastc and p >= 1:
                        for tb4 in range(4):
                            for half in range(2):
                                fillers.append(
                                    lambda t=tb4, hf=half, pp_=p - 1:
                                    cpart(t, hf, pp_))
                    ps_pv = [
                        psV.tile([128, 512], f32, tag="pv", name=f"pspv{h2}")
                        for h2 in range(2)
                    ]
                    prev = None
                    pw = 1024.0
                    for pti in range(nkb // 2):
                        kb0, kb1 = 2 * pti, 2 * pti + 1
                        d0 = kb0 * 128 - j * 512
                        d1 = d0 + 128
                        qlo0, qlo1 = max(d0, 0), max(d1, 0)
                        w0, w1 = 512 - qlo0, 512 - qlo1
                        # one psum tile per key block holding BOTH h2 halves
                        # (h2=1 at column 512) so the paired score matmuls
                        # release together and issue back-to-back
                        pss = [
                            psS.tile([128, 1024], f32, tag="s", name=f"pss{kk}")
                            for kk in range(2)
                        ]
                        for kk, (kb, wd, ql) in enumerate(
                                ((kb0, w0, qlo0), (kb1, w1, qlo1))):
                            for h2 in range(2):
                                hp = h2 * 64
                                nc.tensor.matmul(
                                    pss[kk][:, h2 * 512:h2 * 512 + wd],
                                    lhsT=qkT[hp:hp + 64, 3 + p,
                                             kb * 128:(kb + 1) * 128],
                                    rhs=qkT[hp:hp + 64, p,
                                            j * 512 + ql:(j + 1) * 512],
                                    start=True,
                                    stop=True,
                                )
                        cur = []
                        for kk, (kb, wd, dd) in enumerate(
                                ((kb0, w0, d0), (kb1, w1, d1))):
                            pt = ptp.tile([128, 1024], bf16, tag=f"pt{kk}",
                                          name=f"pt{kk}")
                            if wd == 512:
                                nc.scalar.activation(
                                    pt, pss[kk], AF.Exp, scale=0.125)
                            else:
                                for h2 in range(2):
                                    nc.scalar.activation(
                                        pt[:, h2 * 512:h2 * 512 + wd],
                                        pss[kk][:, h2 * 512:h2 * 512 + wd],
                                        AF.Exp, scale=0.125,
                                    )
                            if dd >= 0:
                                for h2 in range(2):
                                    nc.gpsimd.affine_select(
                                        out=pt[:, h2 * 512:h2 * 512 + 128],
                                        in_=pt[:, h2 * 512:h2 * 512 + 128],
                                        compare_op=ALU.is_ge, fill=0.0, base=0,
                                        pattern=[[1, 128]],
                                        channel_multiplier=-1,
                                    )
                            cur.append((pt, wd))
                        # V for these key blocks (consumed by NEXT pti's PV):
                        # emitted after the scores so the exp stream is never
                        # blocked behind V-proj (which gates on the wv DMA)
                        debt[0] -= emit_v(kb0)
                        debt[0] -= emit_v(kb1)
                        # deficit: ACT exp time minus attention PE time this pti
                        sw = w0 + w1
                        if prev is not None:
                            emit_pv(nc, prev, ps_pv, p, pti - 1, j, nkb)
                            pump(1.25 * sw + 358 - 0.833 * pw)
                        else:
                            pump(1.25 * sw + 358)
                        pw = sw
                        prev = cur
                    emit_pv(nc, prev, ps_pv, p, nkb // 2 - 1, j, nkb)
                    # normalization is deferred one pair (flushed after the
                    # NEXT pair's jit projections) so its vector-queue ops
                    # never block the qkT casts the next scores need
                    norm_pending[0] = (ps_pv, ot_map[j], p)
                pending = (j, ot_map[j])

            flush_norm()
            for f in cfill:
                f()
            for f in fillers:
                f()
            for tb4 in range(4):
                for half in range(2):
                    cpart(tb4, half, 2)

    nc.compile()
    return nc


def _bf16(a):
    import ml_dtypes
    return np.ascontiguousarray(a).astype(ml_dtypes.bfloat16)


def _shard_inputs(x, W_attn, b_attn, W_proj):
    in_maps = []
    for c in range(8):
        b, hg = c // 2, c % 2
        q0, k0, v0 = hg * CH, C + hg * CH, 2 * C + hg * CH
        # per-head bqK column: (Wk_h @ bq_h) -> scores bias via exp-fold
        bcols = np.stack(
            [
                W_attn[:, k0 + h * D:k0 + (h + 1) * D]
                @ b_attn[q0 + h * D:q0 + (h + 1) * D]
                for h in range(NH)
            ],
            axis=1,
        )  # [C, 6]
        in_maps.append({
            "xt": _bf16(x[b].T),
            "wqk": _bf16(np.concatenate(
                [W_attn[:, q0:q0 + CH], W_attn[:, k0:k0 + CH]], axis=1)),
            "wv": _bf16(np.concatenate(
                [W_attn[:, v0:v0 + CH], bcols], axis=1)),
            "wp": _bf16(
                W_proj[hg * CH:(hg + 1) * CH, :]
                .reshape(NPAIR, 128, C)
                .transpose(1, 0, 2)
                .reshape(128, NPAIR * C)
            ),
        })
    return in_maps


def kernel(x, W_attn, b_attn, W_proj, b_proj, _trace=False):
    from concourse.bass_utils import run_bass_kernel_spmd

    x = np.asarray(x, dtype=np.float32)
    W_attn = np.asarray(W_attn, dtype=np.float32)
    b_attn = np.asarray(b_attn, dtype=np.float32)
    W_proj = np.asarray(W_proj, dtype=np.float32)
    b_proj = np.asarray(b_proj, dtype=np.float32)

    if "nc" not in _CACHE:
        _CACHE["nc"] = _build_nc()
    nc = _CACHE["nc"]

    in_maps = _shard_inputs(x, W_attn, b_attn, W_proj)
    res = run_bass_kernel_spmd(nc, in_maps, list(range(8)), trace=_trace)
    _CACHE["last_result"] = res

    # V-bias contribution is a constant row: b_eff = b_proj + b_v @ W_proj
    b_eff = b_proj + b_attn[2 * C:] @ W_proj
    out = np.empty((B, T, C), dtype=np.float32)
    for b in range(B):
        out[b] = res.results[2 * b]["out"] + res.results[2 * b + 1]["out"] + b_eff
    return out


# revision 25
# speedup vs baseline: 1.1737x; 1.0182x over previous
"""Causal self-attention (B=4, T=2048, C=768, H=12) on 8 TRN2 NeuronCores.

Sharding: (batch x head-half). Core c handles batch b = c//2 and heads
hg*6..hg*6+5 where hg = c%2. Each core computes the qkv projection for its
1152 W_attn columns, causal attention for its 6 heads, and a partial
c_proj using its 384 rows of W_proj. Host sums the pair partials + b_eff.

v3 structure (software-pipelined single stream):
- Query chunks processed in order [0, 3, 2, 1] so the exp-heavy chunks are
  not last (ScalarE exp is the secondary bottleneck; the last chunk's exp
  tail would leave the PE idle).
- Q-proj / K-proj / V-proj / c_proj are decoupled into small work items
  that are emitted just-in-time before the attention pti that needs them,
  or earlier as "fillers" paced into the ACT-bound attention inner loop so
  the PE never stalls while ScalarE streams exp.
- Score matmuls of a head pair are emitted back-to-back (h2=0 on PE rows
  0-63, h2=1 on rows 64-127) so they run concurrently in the PE array.
- Bias algebra: K-projection bias dropped (softmax-invariant), V bias
  folded host-side into b_eff = b_proj + b_v @ W_proj, Q bias enters as
  exp(bqK/8) folded into the PV stationary operand.
- PV stationary carries the scaled-ones column (row 64 of the PV
  accumulator = softmax denominator).
- c_proj is split into two 384-wide halves (1 PSUM bank each) and deferred
  one chunk, emitted as filler work.
"""

import sys

import numpy as np

try:
    import concourse  # noqa: F401
except ImportError:
    sys.path.insert(0, "/opt/trn_rl_repo")

B, T, C, H, D = 4, 2048, 768, 12, 64
NH = H // 2          # 6 heads per core
CH = NH * D          # 384 channels per core
NCB = C // 128       # 6 contraction blocks
NTB = T // 128       # 16 t-blocks
NQC = T // 512       # 4 query chunks
NPAIR = NH // 2      # 3 head pairs
VW2 = D + 2          # 66: [V(64), eb, pad] per head (col 64 = eb = exp(bqK/8))
VROW = NH * VW2      # 396
CORDER = [0, 3, 2, 1]

_CACHE = {}


def _build_nc():
    from concourse import bacc, mybir, tile

    f32 = mybir.dt.float32
    bf16 = mybir.dt.bfloat16
    AF = mybir.ActivationFunctionType
    ALU = mybir.AluOpType

    nc = bacc.Bacc("TRN2", target_bir_lowering=False, debug=False, num_devices=8)

    xt_d = nc.dram_tensor("xt", [C, T], bf16, kind="ExternalInput")
    wqk_d = nc.dram_tensor("wqk", [C, 2 * CH], bf16, kind="ExternalInput")
    wv_d = nc.dram_tensor("wv", [C, CH + NH], bf16, kind="ExternalInput")
    wp_d = nc.dram_tensor("wp", [128, NPAIR * C], bf16, kind="ExternalInput")
    out_d = nc.dram_tensor("out", [T, C], f32, kind="ExternalOutput")

    with tile.TileContext(nc) as tc:
        with (
            tc.tile_pool(name="const", bufs=1) as cp,
            tc.tile_pool(name="wk", bufs=3) as wk,
            tc.tile_pool(name="pt", bufs=3) as ptp,
            tc.tile_pool(name="ot", bufs=2) as otp,
            tc.tile_pool(name="outs", bufs=2) as osp,
            tc.tile_pool(name="ps", bufs=2, space="PSUM") as psS,
            tc.tile_pool(name="pj", bufs=2, space="PSUM") as psP,
            tc.tile_pool(name="pv", bufs=2, space="PSUM") as psV,
        ):
            # ---- resident inputs (full-width rows: max DMA run length) ----
            xt_r = xt_d.rearrange("(n p) m -> n p m", p=128)
            wqk_r = wqk_d.rearrange("(n p) m -> n p m", p=128)
            wv_r = wv_d.rearrange("(n p) m -> n p m", p=128)
            xt_t, wqk_t, wv_t = [], [], []
            for ci in range(NCB):
                t_ = cp.tile([128, T], bf16, tag=f"xt{ci}", name=f"xt{ci}")
                nc.sync.dma_start(out=t_, in_=xt_r[ci])
                xt_t.append(t_)
                t_ = cp.tile([128, 2 * CH], bf16, tag=f"wqk{ci}", name=f"wqk{ci}")
                nc.sync.dma_start(out=t_, in_=wqk_r[ci])
                wqk_t.append(t_)
            for ci in range(NCB):
                t_ = cp.tile([128, CH + NH], bf16, tag=f"wv{ci}", name=f"wv{ci}")
                nc.sync.dma_start(out=t_, in_=wv_r[ci])
                wv_t.append(t_)
            wp_sb = cp.tile([128, NPAIR, C], bf16, tag="wp", name="wp")
            nc.sync.dma_start(out=wp_sb, in_=wp_d.rearrange("p (n m) -> p n m", n=NPAIR))

            qkT = cp.tile([128, 6, T], bf16, tag="qkT", name="qkT")  # 0-2: Q, 3-5: K
            v1 = cp.tile([128, NTB, VROW], bf16, tag="v1", name="v1")
            v1_4d = v1.rearrange("p n (h e) -> p n h e", e=VW2)

            # ---------- work items ----------
            done_qk = {}       # (co, tc) -> True   co 0-2 Q-pair, 3-5 K-pair
            done_v = {}        # tb -> True

            def emit_proj(co, tcn):
                """Q or K projection for pair-column co, token chunk tcn."""
                if done_qk.get((co, tcn)):
                    return 0
                done_qk[(co, tcn)] = True
                ps = psP.tile([128, 512], f32, tag="pj", name="pspj")
                for ci in range(NCB):
                    nc.tensor.matmul(
                        ps,
                        lhsT=wqk_t[ci][:, co * 128:(co + 1) * 128],
                        rhs=xt_t[ci][:, tcn * 512:(tcn + 1) * 512],
                        start=(ci == 0),
                        stop=(ci == NCB - 1),
                    )
                nc.vector.tensor_copy(qkT[:, co, tcn * 512:(tcn + 1) * 512], ps)
                return 1450

            def emit_v(tb):
                """V (+ bqK) projection for key t-block tb."""
                if done_v.get(tb):
                    return 0
                done_v[tb] = True
                psv = psP.tile([128, 512], f32, tag="pj", name="pspj")
                for ci in range(NCB):
                    nc.tensor.matmul(
                        psv[:, 0:CH + NH],
                        lhsT=xt_t[ci][:, tb * 128:(tb + 1) * 128],
                        rhs=wv_t[ci],
                        start=(ci == 0),
                        stop=(ci == NCB - 1),
                    )
                eb = wk.tile([128, NH], f32, tag="eb", name="eb")
                nc.scalar.activation(eb, psv[:, CH:CH + NH], AF.Exp, scale=0.125)
                eb3 = eb.rearrange("p (h o) -> p h o", o=1)
                nc.vector.tensor_mul(
                    v1_4d[:, tb, :, 0:D],
                    psv[:, 0:CH].rearrange("p (h e) -> p h e", e=D),
                    eb3.to_broadcast([128, NH, D]),
                )
                nc.vector.tensor_copy(v1_4d[:, tb, :, D:D + 1], eb3)
                return 1150

            def emit_cproj_half(c, tb4, half, ot_tiles):
                """c_proj for t-block c*4+tb4, output columns half*384:+384."""
                tb = c * 4 + tb4
                pp = psP.tile([128, 512], f32, tag="pj", name="pspj")
                for p in range(NPAIR):
                    nc.tensor.matmul(
                        pp[:, 0:CH],
                        lhsT=ot_tiles[p][:, tb4 * 128:(tb4 + 1) * 128],
                        rhs=wp_sb[:, p, half * CH:(half + 1) * CH],
                        start=(p == 0),
                        stop=(p == NPAIR - 1),
                    )
                ost = osp.tile([128, CH], f32, tag=f"ost{half}", name="ost")
                nc.vector.tensor_copy(ost, pp[:, 0:CH])
                nc.sync.dma_start(
                    out=out_d[tb * 128:(tb + 1) * 128, half * CH:(half + 1) * CH],
                    in_=ost,
                )
                return 680

            cfill = []         # c_proj closures (must drain one chunk ahead)
            fillers = []       # proj/V closures returning pe-ns
            debt = [0.0]
            ot_map = {}        # chunk -> [otpair per pair]
            norm_pending = [None]

            def flush_norm():
                if norm_pending[0] is None:
                    return
                ps_pv, ots, p = norm_pending[0]
                norm_pending[0] = None
                otpair = otp.tile([128, 512], bf16, tag=f"ot{p}",
                                  name=f"ot{p}")
                for h2 in range(2):
                    # sums live at partition 64; custom-DVE ops misread
                    # base-64 APs, so standard-copy to partition 0 first
                    sums_sb = wk.tile([1, 512], f32, tag="sums", name="sums")
                    nc.vector.tensor_copy(sums_sb, ps_pv[h2][D:D + 1, :])
                    rb1 = wk.tile([1, 512], f32, tag="rb1", name="rb1")
                    nc.vector.reciprocal_approx_fast(rb1, sums_sb)
                    rbb = wk.tile([64, 512], f32, tag="rbb", name="rbb")
                    nc.gpsimd.partition_broadcast(rbb, rb1)
                    nc.vector.tensor_mul(
                        otpair[h2 * 64:(h2 + 1) * 64, :],
                        ps_pv[h2][0:D, :],
                        rbb,
                    )
                ots[p] = otpair

            def pump(ns):
                debt[0] += ns
                while debt[0] > 0 and (cfill or fillers):
                    q = cfill if cfill else fillers
                    debt[0] -= q.pop(0)()

            def emit_pv(nc_, pts, ps_pv, p, pti, j, nkb):
                for half, kb in ((0, 2 * pti), (1, 2 * pti + 1)):
                    pt, wd = pts[half]
                    qlo = 512 - wd
                    for h2 in range(2):
                        nc_.tensor.matmul(
                            ps_pv[h2][0:1 + D, qlo:512],
                            lhsT=v1_4d[:, kb, 2 * p + h2, 0:1 + D],
                            rhs=pt[:, h2 * 512:h2 * 512 + wd],
                            start=(kb == 0),
                            stop=(kb == nkb - 1),
                        )

            # ---------- main stream ----------
            pending = None     # (chunk, ot_tiles) whose c_proj is deferred
            for ic, j in enumerate(CORDER):
                nkb = 4 * (j + 1)
                # c_proj fillers of the chunk-before-last MUST be fully
                # emitted before this chunk's normalization recycles the ot
                # buffers (otp bufs=2), else PE-queue/WAR cycle -> deadlock
                for f in cfill:
                    f()
                cfill.clear()
                # queue fillers: deferred c_proj, then next chunk's prereqs
                if pending is not None:
                    pc, pots = pending
                    for tb4 in range(4):
                        for half in range(2):
                            cfill.append(
                                lambda c=pc, t=tb4, hf=half, o=pots:
                                emit_cproj_half(c, t, hf, o)
                            )
                    pending = None
                if ic + 1 < len(CORDER):
                    cn = CORDER[ic + 1]
                    for p in range(NPAIR):
                        for tcn in range(cn + 1):
                            if not done_qk.get((3 + p, tcn)):
                                fillers.append(
                                    lambda co=3 + p, t=tcn: emit_proj(co, t))
                        if not done_qk.get((p, cn)):
                            fillers.append(lambda co=p, t=cn: emit_proj(co, t))
                    for tb in range(4 * (cn + 1)):
                        if not done_v.get(tb):
                            fillers.append(lambda t=tb: emit_v(t))

                ot_map[j] = [None] * NPAIR
                for p in range(NPAIR):
                    # jit prereqs for this pair
                    debt[0] -= emit_proj(p, j)
                    for tcn in range(j + 1):
                        debt[0] -= emit_proj(3 + p, tcn)
                    flush_norm()
                    ps_pv = [
                        psV.tile([128, 512], f32, tag="pv", name=f"pspv{h2}")
                        for h2 in range(2)
                    ]
                    prev = None
                    pw = 1024.0
                    for pti in range(nkb // 2):
                        kb0, kb1 = 2 * pti, 2 * pti + 1
                        d0 = kb0 * 128 - j * 512
                        d1 = d0 + 128
                        qlo0, qlo1 = max(d0, 0), max(d1, 0)
                        w0, w1 = 512 - qlo0, 512 - qlo1
                        # one psum tile per key block holding BOTH h2 halves
                        # (h2=1 at column 512) so the paired score matmuls
                        # release together and issue back-to-back
                        pss = [
                            psS.tile([128, 1024], f32, tag="s", name=f"pss{kk}")
                            for kk in range(2)
                        ]
                        for kk, (kb, wd, ql) in enumerate(
                                ((kb0, w0, qlo0), (kb1, w1, qlo1))):
                            for h2 in range(2):
                                hp = h2 * 64
                                nc.tensor.matmul(
                                    pss[kk][:, h2 * 512:h2 * 512 + wd],
                                    lhsT=qkT[hp:hp + 64, 3 + p,
                                             kb * 128:(kb + 1) * 128],
                                    rhs=qkT[hp:hp + 64, p,
                                            j * 512 + ql:(j + 1) * 512],
                                    start=True,
                                    stop=True,
                                )
                        cur = []
                        for kk, (kb, wd, dd) in enumerate(
                                ((kb0, w0, d0), (kb1, w1, d1))):
                            pt = ptp.tile([128, 1024], bf16, tag=f"pt{kk}",
                                          name=f"pt{kk}")
                            if wd == 512:
                                nc.scalar.activation(
                                    pt, pss[kk], AF.Exp, scale=0.125)
                            else:
                                for h2 in range(2):
                                    nc.scalar.activation(
                                        pt[:, h2 * 512:h2 * 512 + wd],
                                        pss[kk][:, h2 * 512:h2 * 512 + wd],
                                        AF.Exp, scale=0.125,
                                    )
                            if dd >= 0:
                                for h2 in range(2):
                                    nc.gpsimd.affine_select(
                                        out=pt[:, h2 * 512:h2 * 512 + 128],
                                        in_=pt[:, h2 * 512:h2 * 512 + 128],
                                        compare_op=ALU.is_ge, fill=0.0, base=0,
                                        pattern=[[1, 128]],
                                        channel_multiplier=-1,
                                    )
                            cur.append((pt, wd))
                        # V for these key blocks (consumed by NEXT pti's PV):
                        # emitted after the scores so the exp stream is never
                        # blocked behind V-proj (which gates on the wv DMA)
                        debt[0] -= emit_v(kb0)
                        debt[0] -= emit_v(kb1)
                        # deficit: ACT exp time minus attention PE time this pti
                        sw = w0 + w1
                        if prev is not None:
                            emit_pv(nc, prev, ps_pv, p, pti - 1, j, nkb)
                            pump(1.25 * sw + 358 - 0.833 * pw)
                        else:
                            pump(1.25 * sw + 358)
                        pw = sw
                        prev = cur
                    emit_pv(nc, prev, ps_pv, p, nkb // 2 - 1, j, nkb)
                    # normalization is deferred one pair (flushed after the
                    # NEXT pair's jit projections) so its vector-queue ops
                    # never block the qkT casts the next scores need
                    norm_pending[0] = (ps_pv, ot_map[j], p)
                pending = (j, ot_map[j])

            # drain leftovers, then the last chunk's c_proj
            flush_norm()
            for f in cfill:
                f()
            for f in fillers:
                f()
            pc, pots = pending
            for tb4 in range(4):
                for half in range(2):
                    emit_cproj_half(pc, tb4, half, pots)

    nc.compile()
    return nc


def _bf16(a):
    import ml_dtypes
    return np.ascontiguousarray(a).astype(ml_dtypes.bfloat16)


def _shard_inputs(x, W_attn, b_attn, W_proj):
    in_maps = []
    for c in range(8):
        b, hg = c // 2, c % 2
        q0, k0, v0 = hg * CH, C + hg * CH, 2 * C + hg * CH
        # per-head bqK column: (Wk_h @ bq_h) -> scores bias via exp-fold
        bcols = np.stack(
            [
                W_attn[:, k0 + h * D:k0 + (h + 1) * D]
                @ b_attn[q0 + h * D:q0 + (h + 1) * D]
                for h in range(NH)
            ],
            axis=1,
        )  # [C, 6]
        in_maps.append({
            "xt": _bf16(x[b].T),
            "wqk": _bf16(np.concatenate(
                [W_attn[:, q0:q0 + CH], W_attn[:, k0:k0 + CH]], axis=1)),
            "wv": _bf16(np.concatenate(
                [W_attn[:, v0:v0 + CH], bcols], axis=1)),
            "wp": _bf16(
                W_proj[hg * CH:(hg + 1) * CH, :]
                .reshape(NPAIR, 128, C)
                .transpose(1, 0, 2)
                .reshape(128, NPAIR * C)
            ),
        })
    return in_maps


def kernel(x, W_attn, b_attn, W_proj, b_proj, _trace=False):
    from concourse.bass_utils import run_bass_kernel_spmd

    x = np.asarray(x, dtype=np.float32)
    W_attn = np.asarray(W_attn, dtype=np.float32)
    b_attn = np.asarray(b_attn, dtype=np.float32)
    W_proj = np.asarray(W_proj, dtype=np.float32)
    b_proj = np.asarray(b_proj, dtype=np.float32)

    if "nc" not in _CACHE:
        _CACHE["nc"] = _build_nc()
    nc = _CACHE["nc"]

    in_maps = _shard_inputs(x, W_attn, b_attn, W_proj)
    res = run_bass_kernel_spmd(nc, in_maps, list(range(8)), trace=_trace)
    _CACHE["last_result"] = res

    # V-bias contribution is a constant row: b_eff = b_proj + b_v @ W_proj
    b_eff = b_proj + b_attn[2 * C:] @ W_proj
    out = np.empty((B, T, C), dtype=np.float32)
    for b in range(B):
        out[b] = res.results[2 * b]["out"] + res.results[2 * b + 1]["out"] + b_eff
    return out


# revision 27
# speedup vs baseline: 1.1800x; 1.0054x over previous
"""Causal self-attention (B=4, T=2048, C=768, H=12) on 8 TRN2 NeuronCores.

Sharding: (batch x head-half). Core c handles batch b = c//2 and heads
hg*6..hg*6+5 where hg = c%2. Each core computes the qkv projection for its
1152 W_attn columns, causal attention for its 6 heads, and a partial
c_proj using its 384 rows of W_proj. Host sums the pair partials + b_eff.

v3 structure (software-pipelined single stream):
- Query chunks processed in order [0, 3, 2, 1] so the exp-heavy chunks are
  not last (ScalarE exp is the secondary bottleneck; the last chunk's exp
  tail would leave the PE idle).
- Q-proj / K-proj / V-proj / c_proj are decoupled into small work items
  that are emitted just-in-time before the attention pti that needs them,
  or earlier as "fillers" paced into the ACT-bound attention inner loop so
  the PE never stalls while ScalarE streams exp.
- Score matmuls of a head pair are emitted back-to-back (h2=0 on PE rows
  0-63, h2=1 on rows 64-127) so they run concurrently in the PE array.
- Bias algebra: K-projection bias dropped (softmax-invariant), V bias
  folded host-side into b_eff = b_proj + b_v @ W_proj, Q bias enters as
  exp(bqK/8) folded into the PV stationary operand.
- PV stationary carries the scaled-ones column (row 64 of the PV
  accumulator = softmax denominator).
- c_proj is split into two 384-wide halves (1 PSUM bank each) and deferred
  one chunk, emitted as filler work.
"""

import sys

import numpy as np

try:
    import concourse  # noqa: F401
except ImportError:
    sys.path.insert(0, "/opt/trn_rl_repo")

B, T, C, H, D = 4, 2048, 768, 12, 64
NH = H // 2          # 6 heads per core
CH = NH * D          # 384 channels per core
NCB = C // 128       # 6 contraction blocks
NTB = T // 128       # 16 t-blocks
NQC = T // 512       # 4 query chunks
NPAIR = NH // 2      # 3 head pairs
VW2 = D + 2          # 66: [V(64), eb, pad] per head (col 64 = eb = exp(bqK/8))
VROW = NH * VW2      # 396
CORDER = [0, 3, 2, 1]

_CACHE = {}


def _build_nc():
    from concourse import bacc, mybir, tile

    f32 = mybir.dt.float32
    bf16 = mybir.dt.bfloat16
    AF = mybir.ActivationFunctionType
    ALU = mybir.AluOpType

    nc = bacc.Bacc("TRN2", target_bir_lowering=False, debug=False, num_devices=8)

    xt_d = nc.dram_tensor("xt", [C, T], bf16, kind="ExternalInput")
    wqk_d = nc.dram_tensor("wqk", [C, 2 * CH], bf16, kind="ExternalInput")
    wv_d = nc.dram_tensor("wv", [C, CH + NH], bf16, kind="ExternalInput")
    wp_d = nc.dram_tensor("wp", [128, NPAIR * C], bf16, kind="ExternalInput")
    out_d = nc.dram_tensor("out", [T, C], f32, kind="ExternalOutput")

    with tile.TileContext(nc) as tc:
        with (
            tc.tile_pool(name="const", bufs=1) as cp,
            tc.tile_pool(name="wk", bufs=3) as wk,
            tc.tile_pool(name="pt", bufs=4) as ptp,
            tc.tile_pool(name="ot", bufs=2) as otp,
            tc.tile_pool(name="outs", bufs=2) as osp,
            tc.tile_pool(name="ps", bufs=2, space="PSUM") as psS,
            tc.tile_pool(name="pj", bufs=2, space="PSUM") as psP,
            tc.tile_pool(name="pv", bufs=2, space="PSUM") as psV,
        ):
            # ---- resident inputs (full-width rows: max DMA run length) ----
            xt_r = xt_d.rearrange("(n p) m -> n p m", p=128)
            wqk_r = wqk_d.rearrange("(n p) m -> n p m", p=128)
            wv_r = wv_d.rearrange("(n p) m -> n p m", p=128)
            xt_t, wqk_t, wv_t = [], [], []
            for ci in range(NCB):
                t_ = cp.tile([128, 2 * CH], bf16, tag=f"wqk{ci}", name=f"wqk{ci}")
                nc.sync.dma_start(out=t_, in_=wqk_r[ci])
                wqk_t.append(t_)
            for ci in range(NCB):
                t_ = cp.tile([128, T], bf16, tag=f"xt{ci}", name=f"xt{ci}")
                nc.sync.dma_start(out=t_, in_=xt_r[ci])
                xt_t.append(t_)
            for ci in range(NCB):
                t_ = cp.tile([128, CH + NH], bf16, tag=f"wv{ci}", name=f"wv{ci}")
                nc.sync.dma_start(out=t_, in_=wv_r[ci])
                wv_t.append(t_)
            wp_sb = cp.tile([128, NPAIR, C], bf16, tag="wp", name="wp")
            nc.sync.dma_start(out=wp_sb, in_=wp_d.rearrange("p (n m) -> p n m", n=NPAIR))

            # causal 0/1 mask for diagonal 128x128 blocks, built once so the
            # per-block masking runs on VectorE (gpsimd's prefix-ordered queue
            # otherwise gates pt reuse behind slow partition_broadcasts)
            tri = cp.tile([128, 128], bf16, tag="tri", name="tri")
            nc.gpsimd.memset(tri, 1.0)
            nc.gpsimd.affine_select(
                out=tri, in_=tri, compare_op=ALU.is_ge, fill=0.0, base=0,
                pattern=[[1, 128]], channel_multiplier=-1,
            )

            qkT = cp.tile([128, 6, T], bf16, tag="qkT", name="qkT")  # 0-2: Q, 3-5: K
            v1 = cp.tile([128, NTB, VROW], bf16, tag="v1", name="v1")
            v1_4d = v1.rearrange("p n (h e) -> p n h e", e=VW2)

            # ---------- work items ----------
            done_qk = {}       # (co, tc) -> True   co 0-2 Q-pair, 3-5 K-pair
            done_v = {}        # tb -> True

            def emit_proj(co, tcn):
                """Q or K projection for pair-column co, token chunk tcn."""
                if done_qk.get((co, tcn)):
                    return 0
                done_qk[(co, tcn)] = True
                ps = psP.tile([128, 512], f32, tag="pj", name="pspj")
                for ci in range(NCB):
                    nc.tensor.matmul(
                        ps,
                        lhsT=wqk_t[ci][:, co * 128:(co + 1) * 128],
                        rhs=xt_t[ci][:, tcn * 512:(tcn + 1) * 512],
                        start=(ci == 0),
                        stop=(ci == NCB - 1),
                    )
                nc.vector.tensor_copy(qkT[:, co, tcn * 512:(tcn + 1) * 512], ps)
                return 1450

            def emit_v(tb):
                """V (+ bqK) projection for key t-block tb."""
                if done_v.get(tb):
                    return 0
                done_v[tb] = True
                psv = psP.tile([128, 512], f32, tag="pj", name="pspj")
                for ci in range(NCB):
                    nc.tensor.matmul(
                        psv[:, 0:CH + NH],
                        lhsT=xt_t[ci][:, tb * 128:(tb + 1) * 128],
                        rhs=wv_t[ci],
                        start=(ci == 0),
                        stop=(ci == NCB - 1),
                    )
                eb = wk.tile([128, NH], f32, tag="eb", name="eb")
                nc.scalar.activation(eb, psv[:, CH:CH + NH], AF.Exp, scale=0.125)
                eb3 = eb.rearrange("p (h o) -> p h o", o=1)
                nc.vector.tensor_mul(
                    v1_4d[:, tb, :, 0:D],
                    psv[:, 0:CH].rearrange("p (h e) -> p h e", e=D),
                    eb3.to_broadcast([128, NH, D]),
                )
                nc.vector.tensor_copy(v1_4d[:, tb, :, D:D + 1], eb3)
                return 1150

            def emit_cproj_half(c, tb4, half, ot_tiles):
                """c_proj for t-block c*4+tb4, output columns half*384:+384."""
                tb = c * 4 + tb4
                pp = psP.tile([128, 512], f32, tag="pj", name="pspj")
                for p in range(NPAIR):
                    nc.tensor.matmul(
                        pp[:, 0:CH],
                        lhsT=ot_tiles[p][:, tb4 * 128:(tb4 + 1) * 128],
                        rhs=wp_sb[:, p, half * CH:(half + 1) * CH],
                        start=(p == 0),
                        stop=(p == NPAIR - 1),
                    )
                ost = osp.tile([128, CH], f32, tag=f"ost{half}", name="ost")
                nc.vector.tensor_copy(ost, pp[:, 0:CH])
                nc.sync.dma_start(
                    out=out_d[tb * 128:(tb + 1) * 128, half * CH:(half + 1) * CH],
                    in_=ost,
                )
                return 680

            cfill = []         # c_proj closures (must drain one chunk ahead)
            fillers = []       # proj/V closures returning pe-ns
            debt = [0.0]
            ot_map = {}        # chunk -> [otpair per pair]
            norm_pending = [None]

            def flush_norm():
                if norm_pending[0] is None:
                    return
                ps_pv, ots, p = norm_pending[0]
                norm_pending[0] = None
                otpair = otp.tile([128, 512], bf16, tag=f"ot{p}",
                                  name=f"ot{p}")
                for h2 in range(2):
                    # sums live at partition 64; custom-DVE ops misread
                    # base-64 APs, so standard-copy to partition 0 first
                    sums_sb = wk.tile([1, 512], f32, tag="sums", name="sums")
                    nc.vector.tensor_copy(sums_sb, ps_pv[h2][D:D + 1, :])
                    rb1 = wk.tile([1, 512], f32, tag="rb1", name="rb1")
                    nc.vector.reciprocal_approx_fast(rb1, sums_sb)
                    rbb = wk.tile([64, 512], f32, tag="rbb", name="rbb")
                    nc.gpsimd.partition_broadcast(rbb, rb1)
                    nc.vector.tensor_mul(
                        otpair[h2 * 64:(h2 + 1) * 64, :],
                        ps_pv[h2][0:D, :],
                        rbb,
                    )
                ots[p] = otpair

            def pump(ns):
                debt[0] += ns
                while debt[0] > 0 and (cfill or fillers):
                    q = cfill if cfill else fillers
                    debt[0] -= q.pop(0)()

            def emit_pv(nc_, pts, ps_pv, p, pti, j, nkb):
                for half, kb in ((0, 2 * pti), (1, 2 * pti + 1)):
                    pt, wd = pts[half]
                    qlo = 512 - wd
                    for h2 in range(2):
                        nc_.tensor.matmul(
                            ps_pv[h2][0:1 + D, qlo:512],
                            lhsT=v1_4d[:, kb, 2 * p + h2, 0:1 + D],
                            rhs=pt[:, h2 * 512:h2 * 512 + wd],
                            start=(kb == 0),
                            stop=(kb == nkb - 1),
                        )

            # ---------- main stream ----------
            pending = None     # (chunk, ot_tiles) whose c_proj is deferred
            for ic, j in enumerate(CORDER):
                nkb = 4 * (j + 1)
                # c_proj fillers of the chunk-before-last MUST be fully
                # emitted before this chunk's normalization recycles the ot
                # buffers (otp bufs=2), else PE-queue/WAR cycle -> deadlock
                for f in cfill:
                    f()
                cfill.clear()
                # queue fillers: deferred c_proj, then next chunk's prereqs
                if pending is not None:
                    pc, pots = pending
                    for tb4 in range(4):
                        for half in range(2):
                            cfill.append(
                                lambda c=pc, t=tb4, hf=half, o=pots:
                                emit_cproj_half(c, t, hf, o)
                            )
                    pending = None
                if ic + 1 < len(CORDER):
                    cn = CORDER[ic + 1]
                    for p in range(NPAIR):
                        for tcn in range(cn + 1):
                            if not done_qk.get((3 + p, tcn)):
                                fillers.append(
                                    lambda co=3 + p, t=tcn: emit_proj(co, t))
                        if not done_qk.get((p, cn)):
                            fillers.append(lambda co=p, t=cn: emit_proj(co, t))
                    for tb in range(4 * (cn + 1)):
                        if not done_v.get(tb):
                            fillers.append(lambda t=tb: emit_v(t))

                ot_map[j] = [None] * NPAIR
                for p in range(NPAIR):
                    # jit prereqs for this pair
                    debt[0] -= emit_proj(p, j)
                    for tcn in range(j + 1):
                        debt[0] -= emit_proj(3 + p, tcn)
                    flush_norm()
                    ps_pv = [
                        psV.tile([128, 512], f32, tag="pv", name=f"pspv{h2}")
                        for h2 in range(2)
                    ]
                    prev = None
                    pw = 1024.0
                    for pti in range(nkb // 2):
                        kb0, kb1 = 2 * pti, 2 * pti + 1
                        d0 = kb0 * 128 - j * 512
                        d1 = d0 + 128
                        qlo0, qlo1 = max(d0, 0), max(d1, 0)
                        w0, w1 = 512 - qlo0, 512 - qlo1
                        # one psum tile per key block holding BOTH h2 halves
                        # (h2=1 at column 512) so the paired score matmuls
                        # release together and issue back-to-back
                        pss = [
                            psS.tile([128, 1024], f32, tag="s", name=f"pss{kk}")
                            for kk in range(2)
                        ]
                        for kk, (kb, wd, ql) in enumerate(
                                ((kb0, w0, qlo0), (kb1, w1, qlo1))):
                            for h2 in range(2):
                                hp = h2 * 64
                                nc.tensor.matmul(
                                    pss[kk][:, h2 * 512:h2 * 512 + wd],
                                    lhsT=qkT[hp:hp + 64, 3 + p,
                                             kb * 128:(kb + 1) * 128],
                                    rhs=qkT[hp:hp + 64, p,
                                            j * 512 + ql:(j + 1) * 512],
                                    start=True,
                                    stop=True,
                                )
                        cur = []
                        for kk, (kb, wd, dd) in enumerate(
                                ((kb0, w0, d0), (kb1, w1, d1))):
                            pt = ptp.tile([128, 1024], bf16, tag=f"pt{kk}",
                                          name=f"pt{kk}")
                            if wd == 512:
                                nc.scalar.activation(
                                    pt, pss[kk], AF.Exp, scale=0.125)
                            else:
                                for h2 in range(2):
                                    nc.scalar.activation(
                                        pt[:, h2 * 512:h2 * 512 + wd],
                                        pss[kk][:, h2 * 512:h2 * 512 + wd],
                                        AF.Exp, scale=0.125,
                                    )
                            if dd >= 0:
                                for h2 in range(2):
                                    nc.vector.tensor_mul(
                                        pt[:, h2 * 512:h2 * 512 + 128],
                                        pt[:, h2 * 512:h2 * 512 + 128],
                                        tri,
                                    )
                            cur.append((pt, wd))
                        # V for these key blocks (consumed by NEXT pti's PV):
                        # emitted after the scores so the exp stream is never
                        # blocked behind V-proj (which gates on the wv DMA)
                        debt[0] -= emit_v(kb0)
                        debt[0] -= emit_v(kb1)
                        # deficit: ACT exp time minus attention PE time this pti
                        sw = w0 + w1
                        if prev is not None:
                            emit_pv(nc, prev, ps_pv, p, pti - 1, j, nkb)
                            pump(1.25 * sw + 358 - 0.833 * pw)
                        else:
                            pump(1.25 * sw + 358)
                        pw = sw
                        prev = cur
                    emit_pv(nc, prev, ps_pv, p, nkb // 2 - 1, j, nkb)
                    # normalization is deferred one pair (flushed after the
                    # NEXT pair's jit projections) so its vector-queue ops
                    # never block the qkT casts the next scores need
                    norm_pending[0] = (ps_pv, ot_map[j], p)
                pending = (j, ot_map[j])

            # drain leftovers, then the last chunk's c_proj
            flush_norm()
            for f in cfill:
                f()
            for f in fillers:
                f()
            pc, pots = pending
            for tb4 in range(4):
                for half in range(2):
                    emit_cproj_half(pc, tb4, half, pots)

    nc.compile()
    return nc


def _bf16(a):
    import ml_dtypes
    return np.ascontiguousarray(a).astype(ml_dtypes.bfloat16)


def _shard_inputs(x, W_attn, b_attn, W_proj):
    in_maps = []
    for c in range(8):
        b, hg = c // 2, c % 2
        q0, k0, v0 = hg * CH, C + hg * CH, 2 * C + hg * CH
        # per-head bqK column: (Wk_h @ bq_h) -> scores bias via exp-fold
        bcols = np.stack(
            [
                W_attn[:, k0 + h * D:k0 + (h + 1) * D]
                @ b_attn[q0 + h * D:q0 + (h + 1) * D]
                for h in range(NH)
            ],
            axis=1,
        )  # [C, 6]
        in_maps.append({
            "xt": _bf16(x[b].T),
            "wqk": _bf16(np.concatenate(
                [W_attn[:, q0:q0 + CH], W_attn[:, k0:k0 + CH]], axis=1)),
            "wv": _bf16(np.concatenate(
                [W_attn[:, v0:v0 + CH], bcols], axis=1)),
            "wp": _bf16(
                W_proj[hg * CH:(hg + 1) * CH, :]
                .reshape(NPAIR, 128, C)
                .transpose(1, 0, 2)
                .reshape(128, NPAIR * C)
            ),
        })
    return in_maps


def kernel(x, W_attn, b_attn, W_proj, b_proj, _trace=False):
    from concourse.bass_utils import run_bass_kernel_spmd

    x = np.asarray(x, dtype=np.float32)
    W_attn = np.asarray(W_attn, dtype=np.float32)
    b_attn = np.asarray(b_attn, dtype=np.float32)
    W_proj = np.asarray(W_proj, dtype=np.float32)
    b_proj = np.asarray(b_proj, dtype=np.float32)

    if "nc" not in _CACHE:
        _CACHE["nc"] = _build_nc()
    nc = _CACHE["nc"]

    in_maps = _shard_inputs(x, W_attn, b_attn, W_proj)
    res = run_bass_kernel_spmd(nc, in_maps, list(range(8)), trace=_trace)
    _CACHE["last_result"] = res

    # V-bias contribution is a constant row: b_eff = b_proj + b_v @ W_proj
    b_eff = b_proj + b_attn[2 * C:] @ W_proj
    out = np.empty((B, T, C), dtype=np.float32)
    for b in range(B):
        out[b] = res.results[2 * b]["out"] + res.results[2 * b + 1]["out"] + b_eff
    return out
